# revision 28
# baseline (speedup 1.0000x reference)
"""Trainium2 Bass kernel for nn_AttentionModule (gated-SE + global attention pooling GNN).

Math (per reference):
  att = tanh(relu(x@w1+b1)@w2+b2); x2 = (1+att)*x = 2*sigmoid(2*(pre+b2))*x
  mean = segment_mean(x2, batch); tg = tanh(mean @ W)
  coef = sigmoid(sum(x2 * tg[batch], -1)); out = segment_sum(coef[:,None]*x2, batch)

Strategy: data-parallel over graphs (512 graphs/core on 4 cores — fewer cores than
the 8 available because per-call dispatch cost scales with core count while device
exec is far below the transfer time; batch is sorted so each core's nodes are
contiguous). Per core, eight 64-graph windows, software-pipelined (p1(w+1) emitted
before p2(w)). Nodes padded so each window has a uniform
block count NBW on every core (SPMD: one program, per-core data). Every 128-node
block touches at most 2 graphs (min graph size > 128) -> per-block *pair* partial
segment sums in block-indexed PSUM columns; host-built 0/1 combine matrices reduce
pairs -> graphs. All data-dependent indexing lives in host-built mask/gather/combine
matrices, never in the program.

The dominant per-call costs in this environment are (a) shipping the input buffers
to the devices (~12.6 GB/s aggregate) and (b) a fixed ~0.7 ms per distinct input
tensor. So ALL inputs are packed into ONE int8 tensor per core: x pre-transposed as
int8 with per-(dim, 128-node-block) bf16 dequant scales, plus bf16 masks/combine
matrices/params, each segment read on device through a bitcast view. The device
dequantizes each x chunk with a cast + broadcast-multiply. It stores
x2' = sigmoid(2 z)*x (= x2/2) in both layouts (x2T from the dequantized xT, x2n via
bf16 PE transposes); the factor 2 is folded into the per-graph mean scales (icm),
the coef sigmoid scale, and a final host-side doubling. Output ships back as bf16
and is upcast to f32 on the host.
"""

import hashlib
import os
from contextlib import ExitStack

import numpy as np

P = 128
D = 128
R = 32
G = 2048
NCORES = 4
GPC = G // NCORES       # graphs per core
WG = 64                 # graphs per window
NW = GPC // WG          # windows per core = 4

_F32 = np.float32


def _bf16():
    import ml_dtypes
    return ml_dtypes.bfloat16


def _pin_neff_cache():
    """Key the NEFF cache by this file's content: the neuron cache hashes the
    HLO without the embedded BIR, so same-shape kernel edits would silently
    reuse a stale NEFF otherwise."""
    try:
        with open(__file__, "rb") as f:
            tag = hashlib.sha256(f.read()).hexdigest()[:16]
        path = f"/tmp/neuron-cache-{tag}"
        os.makedirs(path, exist_ok=True)
        os.environ["NEURON_COMPILE_CACHE_URL"] = path
    except OSError:
        pass  # fall back to the default cache rather than failing the call


def _offsets(NBW, NPW, NPAIR, NCHK):
    """Byte offsets of each segment within the per-core pack (per partition)."""
    off = {}
    o = 0
    for name, nbytes in [
        ("x", NW * NPW),                 # int8, partition = dim
        ("sc", NW * NBW * 2),            # bf16 dequant scales, partition = dim
        ("m2", NW * NPAIR),              # int8 0/1, partition = node-in-block
        ("gm", NW * NPAIR),              # bf16, win pairs folded on partitions
        ("cb", NW * NCHK * WG),          # int8 0/1, partition = pair-row
        ("icm", WG * NW * 2),            # bf16 2/count per (window, graph)
        ("pkb", (R + 3 * P) * 2),        # bf16 params
        ("pkf", 2 * 4),                  # f32 biases
    ]:
        assert o % 4 == 0
        off[name] = o
        o += nbytes
    off["total"] = o
    return off


# ---------------------------------------------------------------- host prep

def _prep(x, batch):
    """Build per-core packed int8 streams + mask/gather/combine matrices."""
    bf16 = _bf16()
    counts = np.bincount(batch, minlength=G).astype(np.int64)
    cum = np.concatenate([[0], np.cumsum(counts)])

    win_rng = []  # (core, w) -> (s, e)
    for c in range(NCORES):
        for w in range(NW):
            glo = c * GPC + w * WG
            win_rng.append((int(cum[glo]), int(cum[glo + WG])))
    max_nodes = max(e - s for s, e in win_rng)
    NBW = (max_nodes + P - 1) // P
    NBW = ((NBW + 63) // 64) * 64          # NPAIR == 2*NBW (all pair cols written)
    assert 2 * NBW <= 512, f"window too large: NBW={NBW}"
    NPW = NBW * P
    NPAIR = 2 * NBW
    NCHK = NPAIR // P

    xs = np.zeros((NCORES, D, NW * NPW), dtype=np.int8)       # transposed int8
    sc = np.ones((NCORES, D, NW * NBW), dtype=bf16)           # dequant scales
    m2 = np.zeros((NCORES, NW, P, NPAIR), dtype=np.int8)
    gm = np.zeros((NCORES, NW, WG, NPAIR), dtype=bf16)
    cb = np.zeros((NCORES, NW, NCHK, P, WG), dtype=np.int8)
    icm = np.zeros((NCORES, NW, WG), dtype=bf16)

    wpb_max = 1
    for c in range(NCORES):
        for w in range(NW):
            s, e = win_rng[c * NW + w]
            n = e - s
            glo = c * GPC + w * WG
            # group-int8 quantize: groups of (128-node block, dim)
            xpad = np.zeros((NPW, D), dtype=_F32)
            xpad[:n] = x[s:e]
            xb = xpad.reshape(NBW, P, D)
            gmax = np.abs(xb).max(axis=1)                      # [NBW, D]
            step = (np.where(gmax > 0, gmax, 1.0) / 127.0).astype(bf16)
            stepf = step.astype(_F32)                          # [NBW, D]
            q = np.rint(xb / stepf[:, None, :]).clip(-127, 127).astype(np.int8)
            xs[c, :, w * NPW:(w + 1) * NPW] = q.reshape(NPW, D).T
            sc[c, :, w * NBW:(w + 1) * NBW] = step.T
            lid = np.full(NPW, -1, dtype=np.int64)
            lid[:n] = batch[s:e] - glo
            # factor 2 of x2 = 2*x2' folded into the per-graph mean scales
            icm[c, w] = (2.0 / np.maximum(counts[glo:glo + WG], 1)).astype(bf16)
            for b in range(NBW):
                ids = lid[b * P:(b + 1) * P]
                uniq = np.unique(ids[ids >= 0])
                wpb_max = max(wpb_max, len(uniq))
                if len(uniq) == 0:
                    gp = [0, 1]
                elif len(uniq) == 1:
                    g0 = int(uniq[0])
                    gp = [g0, g0 + 1 if g0 + 1 < WG else g0 - 1]
                else:
                    gp = [int(uniq[0]), int(uniq[1])]
                for j, gcol in enumerate(gp):
                    sel = ids == gcol
                    if sel.any():
                        m2[c, w, sel, 2 * b + j] = 1
                    gm[c, w, gcol, 2 * b + j] = 1.0
                    pr = 2 * b + j
                    cb[c, w, pr // P, pr % P, gcol] = 1
    assert wpb_max <= 2, f"block spans {wpb_max} graphs; pair assumption violated"
    return xs, sc, m2, gm, cb, icm, NBW, NPW, NPAIR, NCHK


# ---------------------------------------------------------------- program

def _build(NBW, NPW, NPAIR, NCHK, use_b1=False, use_b2=False):
    import concourse.bass as bass_mod
    import concourse.bacc as bacc
    import concourse.tile as tile
    from concourse import mybir
    from concourse.alu_op_type import AluOpType

    f32 = mybir.dt.float32
    bf = mybir.dt.bfloat16
    i8 = mybir.dt.int8
    AF = mybir.ActivationFunctionType
    NGRP = NBW // 16
    OFF = _offsets(NBW, NPW, NPAIR, NCHK)

    nc = bacc.Bacc(enable_partition_id=False)
    packd = nc.dram_tensor("pk", [P, OFF["total"]], i8, kind="ExternalInput")
    outd = nc.dram_tensor("out", [GPC, D], bf, kind="ExternalOutput")

    def seg_bf(name, w, nbytes_w, rows=P):
        o = OFF[name] + w * nbytes_w
        return packd[0:rows, o:o + nbytes_w].bitcast(bf)

    def seg_i8(name, w, nbytes_w):
        o = OFF[name] + w * nbytes_w
        return packd[:, o:o + nbytes_w]

    with tile.TileContext(nc) as tc, ExitStack() as ctx:
        sing = ctx.enter_context(tc.tile_pool(name="sing", bufs=1))
        xqp = ctx.enter_context(tc.tile_pool(name="xqp", bufs=3))
        xrp = ctx.enter_context(tc.tile_pool(name="xrp", bufs=2))
        xtp = ctx.enter_context(tc.tile_pool(name="xtp", bufs=6))
        hsp = ctx.enter_context(tc.tile_pool(name="hsp", bufs=3))
        sgp = ctx.enter_context(tc.tile_pool(name="sgp", bufs=3))
        mkp = ctx.enter_context(tc.tile_pool(name="mkp", bufs=2))
        gbp = ctx.enter_context(tc.tile_pool(name="gbp", bufs=2))
        tgp = ctx.enter_context(tc.tile_pool(name="tgp", bufs=2))
        cbp = ctx.enter_context(tc.tile_pool(name="cbp", bufs=2))
        mds = ctx.enter_context(tc.tile_pool(name="mds", bufs=4))
        ssp = ctx.enter_context(tc.tile_pool(name="ssp", bufs=4))
        scp = ctx.enter_context(tc.tile_pool(name="scp", bufs=2))
        big = ctx.enter_context(tc.tile_pool(name="big", bufs=2))
        # psum pools, 8 banks total: h(1) att(2) xn(2) pair(2) pt(1)
        hpp = ctx.enter_context(tc.tile_pool(name="hpp", bufs=1, space="PSUM"))
        app = ctx.enter_context(tc.tile_pool(name="app", bufs=1, space="PSUM"))
        xnp = ctx.enter_context(tc.tile_pool(name="xnp", bufs=2, space="PSUM"))
        prp = ctx.enter_context(tc.tile_pool(name="prp", bufs=2, space="PSUM"))
        ptp = ctx.enter_context(tc.tile_pool(name="ptp", bufs=1, space="PSUM"))

        pkb = sing.tile([P, R + 3 * P], bf)
        nc.gpsimd.dma_start(out=pkb, in_=seg_bf("pkb", 0, (R + 3 * P) * 2))
        pkf = sing.tile([P, 2], f32)
        nc.gpsimd.dma_start(
            out=pkf, in_=packd[:, OFF["pkf"]:OFF["pkf"] + 8].bitcast(f32))
        icma = sing.tile([P, NW * WG], bf)
        nc.gpsimd.dma_start(out=icma, in_=seg_bf("icm", 0, NW * WG * 2))
        w1s = pkb[:, 0:R]
        w2s = pkb[:, R:R + P]
        idb = pkb[:, R + P:R + 2 * P]
        Ws = pkb[:, R + 2 * P:R + 3 * P]
        b1s = pkf[:, 0:1]
        b2s = pkf[:, 1:2]

        st = {}

        def emit_p1(w):
            s = {}
            s["x2T"] = big.tile([P, NPW], bf, tag="x2T", name="x2T")
            s["x2n"] = big.tile([P, NPW], bf, tag="x2n", name="x2n")
            cbq = cbp.tile([P, NCHK * WG], i8, tag="cbq", name="cbq")
            nc.gpsimd.dma_start(out=cbq, in_=seg_i8("cb", w, NCHK * WG))
            s["cbw"] = cbp.tile([P, NCHK * WG], bf, tag="cb", name="cbw")
            nc.scalar.copy(s["cbw"], cbq)
            # gm: window pairs folded onto partitions 0..63 / 64..127
            go = OFF["gm"] + (w // 2) * NPAIR * 2
            r0 = (w % 2) * WG
            s["gb"] = gbp.tile([WG, NPAIR], bf, tag="gb", name="gb")
            nc.gpsimd.dma_start(
                out=s["gb"], in_=packd[r0:r0 + WG, go:go + NPAIR * 2].bitcast(bf))
            mkq = mkp.tile([P, NPAIR], i8, tag="mkq", name="mkq")
            nc.gpsimd.dma_start(out=mkq, in_=seg_i8("m2", w, NPAIR))
            s["mkb"] = mkp.tile([P, NPAIR], bf, tag="mk", name="mkb")
            nc.scalar.copy(s["mkb"], mkq)
            sct = scp.tile([P, NBW], bf, tag="sc", name="sct")
            nc.gpsimd.dma_start(out=sct, in_=seg_bf("sc", w, NBW * 2))
            x2T, x2n, mkb = s["x2T"], s["x2n"], s["mkb"]
            pair = prp.tile([P, NPAIR], f32, tag="pair")
            s["pair"] = pair
            for g in range(NGRP):
                b0 = g * 16 * P
                xq = xqp.tile([P, 2048], i8, tag="xq")
                co = OFF["x"] + w * NPW + b0
                nc.sync.dma_start(out=xq, in_=packd[:, co:co + 2048])
                xr = xrp.tile([P, 2048], bf, tag="xr")
                nc.scalar.copy(xr, xq)
                xt = xtp.tile([P, 2048], bf, tag="xt")
                scb = sct[:, g * 16:(g + 1) * 16]
                scv = bass_mod.AP(
                    tensor=scb.tensor, offset=scb.offset,
                    ap=[list(scb.ap[0]), list(scb.ap[1]), [0, P]])
                nc.vector.tensor_tensor(
                    xt.rearrange("p (k n) -> p k n", n=P),
                    xr.rearrange("p (k n) -> p k n", n=P),
                    scv, op=AluOpType.mult)
                xts = [xt[:, 512 * sb:512 * sb + 512] for sb in range(4)]
                hps = hpp.tile([P, 512], f32, tag="h")
                for sb in range(4):
                    nc.tensor.matmul(hps[32 * sb:32 * sb + 32, :], lhsT=w1s,
                                     rhs=xts[sb], start=True, stop=True,
                                     tile_position=(0, 32 * sb))
                hs = hsp.tile([P, 512], bf, tag="hs")
                nc.scalar.activation(hs, hps, AF.Relu,
                                     bias=b1s if use_b1 else 0.0)
                for half in range(2):
                    att = app.tile([P, 1024], f32, tag="att")
                    for s2 in range(2):
                        sb = half * 2 + s2
                        nc.tensor.matmul(att[:, 512 * s2:512 * s2 + 512],
                                         lhsT=w2s[32 * sb:32 * sb + 32, :],
                                         rhs=hs[32 * sb:32 * sb + 32, :],
                                         start=True, stop=True,
                                         tile_position=(32 * sb, 0))
                    sg = sgp.tile([P, 1024], bf, tag="sg")
                    nc.scalar.activation(sg, att, AF.Sigmoid,
                                         bias=b2s if use_b2 else 0.0, scale=2.0)
                    c0 = (g * 16 + half * 8) * P
                    nc.vector.tensor_tensor(
                        x2T[:, c0:c0 + 1024], sg,
                        xt[:, 1024 * half:1024 * half + 1024],
                        op=AluOpType.mult,
                    )
                for hf in range(2):
                    xnt = xnp.tile([P, 1024], bf, tag="xn")
                    c0 = (g * 16 + hf * 8) * P
                    for k in range(8):
                        nc.tensor.transpose(
                            xnt[:, 128 * k:128 * k + 128],
                            x2T[:, c0 + 128 * k:c0 + 128 * k + 128],
                            idb)
                    nc.vector.tensor_copy(x2n[:, c0:c0 + 1024], xnt)
                for k in range(16):
                    b = g * 16 + k
                    nc.tensor.matmul(pair[:, 2 * b:2 * b + 2],
                                     lhsT=x2n[:, b * P:b * P + P],
                                     rhs=mkb[:, 2 * b:2 * b + 2],
                                     start=True, stop=True)
            st[w] = s

        def emit_mid(w):
            s = st[w]
            cbw, gb, pair = s["cbw"], s["gb"], s["pair"]
            sps = mds.tile([P, NPAIR], bf, tag="sps")
            nc.vector.tensor_copy(sps, pair)
            mtp = xnp.tile([P, 512], f32, tag="xn")
            for k in range(NCHK):
                tp = ptp.tile([P, 128], f32, tag="pt")
                nc.tensor.matmul(tp, lhsT=sps[:, k * P:(k + 1) * P], rhs=idb,
                                 start=True, stop=True)
                spn = mds.tile([P, 128], bf, tag="spn")
                nc.vector.tensor_copy(spn, tp)
                nc.tensor.matmul(mtp[:, :WG], lhsT=spn,
                                 rhs=cbw[:, k * WG:(k + 1) * WG],
                                 start=(k == 0), stop=(k == NCHK - 1))
            meanT = mds.tile([P, WG], bf, tag="meanT")
            nc.vector.tensor_tensor(meanT, mtp[:, :WG],
                                    icma[:, w * WG:(w + 1) * WG],
                                    op=AluOpType.mult)
            tp2 = ptp.tile([P, 128], f32, tag="pt")
            nc.tensor.matmul(tp2[:WG, :], lhsT=meanT, rhs=Ws, start=True, stop=True)
            tgn = mds.tile([WG, 128], bf, tag="tgn")
            nc.scalar.activation(tgn, tp2[:WG, :], AF.Tanh)
            tp4 = xnp.tile([P, 512], f32, tag="xn")
            nc.tensor.matmul(tp4[:, :NPAIR], lhsT=tgn, rhs=gb, start=True, stop=True)
            tgpair = tgp.tile([P, NPAIR], bf)
            nc.scalar.copy(tgpair, tp4[:, :NPAIR])
            s["tgpair"] = tgpair

        def emit_p2(w):
            s = st[w]
            x2T, x2n, mkb, cbw, tgpair = (s["x2T"], s["x2n"], s["mkb"],
                                          s["cbw"], s["tgpair"])
            opair = prp.tile([P, NPAIR], f32, tag="pair")
            for g4 in range(NGRP // 4):
                bb = g4 * 64          # first block of this 4-group super
                ptt = ptp.tile([P, 128], f32, tag="pt")
                for k in range(64):
                    b = bb + k
                    nc.tensor.matmul(ptt[:, 2 * k:2 * k + 2],
                                     lhsT=x2T[:, b * P:b * P + P],
                                     rhs=tgpair[:, 2 * b:2 * b + 2],
                                     start=True, stop=True)
                tmp = ssp.tile([P, 128], f32, tag="tmp")
                nc.vector.tensor_tensor(tmp, ptt, mkb[:, 2 * bb:2 * bb + 128],
                                        op=AluOpType.mult)
                sred = ssp.tile([P, 64], f32, tag="sred")
                nc.vector.reduce_sum(sred, tmp.rearrange("p (k t) -> p k t", t=2),
                                     axis=mybir.AxisListType.X)
                coef = ssp.tile([P, 64], f32, tag="coef")
                nc.scalar.activation(coef, sred, AF.Sigmoid, scale=2.0)
                cmk = ssp.tile([P, 128], bf, tag="cmk")
                coef_b = bass_mod.AP(
                    tensor=coef.tensor, offset=coef.offset,
                    ap=[list(coef.ap[0]), [list(coef.ap[1])[0], 64], [0, 2]])
                nc.vector.tensor_tensor(
                    cmk.rearrange("p (k t) -> p k t", t=2),
                    mkb[:, 2 * bb:2 * bb + 128].rearrange("p (k t) -> p k t", t=2),
                    coef_b, op=AluOpType.mult)
                for k in range(64):
                    b = bb + k
                    nc.tensor.matmul(opair[:, 2 * b:2 * b + 2],
                                     lhsT=x2n[:, b * P:b * P + P],
                                     rhs=cmk[:, 2 * k:2 * k + 2],
                                     start=True, stop=True)
            outn = xnp.tile([P, 512], f32, tag="xn")
            for k in range(NCHK):
                ops = mds.tile([P, 128], bf, tag="sps")
                nc.vector.tensor_copy(ops, opair[:, k * P:(k + 1) * P])
                tp = ptp.tile([P, 128], f32, tag="pt")
                nc.tensor.matmul(tp, lhsT=ops, rhs=idb, start=True, stop=True)
                opn = mds.tile([P, 128], bf, tag="spn")
                nc.vector.tensor_copy(opn, tp)
                nc.tensor.matmul(outn[:WG, :128], lhsT=cbw[:, k * WG:(k + 1) * WG],
                                 rhs=opn, start=(k == 0), stop=(k == NCHK - 1))
            outs = mds.tile([WG, 128], bf, tag="outs")
            nc.scalar.copy(outs, outn[:WG, :128])
            nc.gpsimd.dma_start(out=outd[w * WG:(w + 1) * WG, :], in_=outs)
            del st[w]

        for w in range(NW):
            emit_p1(w)
            if w > 0:
                emit_p2(w - 1)
            emit_mid(w)
        emit_p2(NW - 1)

    nc.compile()
    return nc


# ---------------------------------------------------------------- driver

def _make_in_maps(inputs):
    bf16 = _bf16()
    x = np.asarray(inputs["x"], _F32)
    batch = np.asarray(inputs["batch"]).astype(np.int64)
    fc_w1 = np.asarray(inputs["fc_w1"], _F32)
    fc_b1 = np.asarray(inputs["fc_b1"], _F32)
    fc_w2 = np.asarray(inputs["fc_w2"], _F32)
    fc_b2 = np.asarray(inputs["fc_b2"], _F32)
    W = np.asarray(inputs["W"], _F32)

    xs, sc, m2, gm, cb, icm, NBW, NPW, NPAIR, NCHK = _prep(x, batch)
    OFF = _offsets(NBW, NPW, NPAIR, NCHK)
    pkb = np.zeros((P, R + 3 * P), dtype=bf16)
    pkb[:, 0:R] = fc_w1.astype(bf16)
    pkb[:, R:R + P] = np.tile(fc_w2, (4, 1)).astype(bf16)
    pkb[:, R + P:R + 2 * P] = np.eye(P, dtype=_F32).astype(bf16)
    pkb[:, R + 2 * P:R + 3 * P] = W.astype(bf16)
    pkf = np.zeros((P, 2), dtype=_F32)
    pkf[:, 0] = np.tile(fc_b1, 4)
    pkf[:, 1] = 2.0 * fc_b2

    in_maps = []
    for c in range(NCORES):
        # gm: fold window pairs onto partitions (w%2 -> rows 0/64)
        gmp = np.zeros((P, NW // 2 * NPAIR), dtype=bf16)
        for w in range(NW):
            gmp[(w % 2) * WG:(w % 2) * WG + WG,
                (w // 2) * NPAIR:(w // 2 + 1) * NPAIR] = gm[c, w]
        # icm replicated along partitions
        icp = np.broadcast_to(icm[c].reshape(1, NW * WG), (P, NW * WG))
        segs = [
            xs[c].view(np.int8),
            sc[c].view(np.int8),
            m2[c].transpose(1, 0, 2).reshape(P, NW * NPAIR),
            gmp.view(np.int8),
            cb[c].transpose(2, 0, 1, 3).reshape(P, NW * NCHK * WG),
            np.ascontiguousarray(icp).view(np.int8),
            pkb.view(np.int8),
            pkf.view(np.int8),
        ]
        pack = np.ascontiguousarray(np.concatenate(segs, axis=1))
        assert pack.shape == (P, OFF["total"]), (pack.shape, OFF["total"])
        in_maps.append({"pk": pack})
    dims = (NBW, NPW, NPAIR, NCHK)
    flags = (bool(np.abs(fc_b1).max() > 0), bool(np.abs(fc_b2).max() > 0))
    return in_maps, dims, flags


def _run(inputs, trace=False):
    import sys
    import time
    if "/opt/trn_rl_repo" not in sys.path:
        sys.path.insert(0, "/opt/trn_rl_repo")
    _pin_neff_cache()
    from concourse.bass_utils import run_bass_kernel_spmd

    in_maps, (NBW, NPW, NPAIR, NCHK), (use_b1, use_b2) = _make_in_maps(inputs)
    nc = _build(NBW, NPW, NPAIR, NCHK, use_b1=use_b1, use_b2=use_b2)
    # one retry: the axon-tunneled devices occasionally drop a call with a
    # transient NRT_EXEC_UNIT_UNRECOVERABLE; a fresh attempt recovers
    last_err = None
    for attempt in range(2):
        try:
            res = run_bass_kernel_spmd(nc, in_maps, core_ids=list(range(NCORES)),
                                       trace=trace)
            break
        except Exception as e:  # noqa: BLE001 - re-raised below on 2nd failure
            last_err = e
            if attempt == 1:
                raise
            time.sleep(5.0)
    out = 2.0 * np.concatenate(
        [np.asarray(r["out"], _F32) for r in res.results], axis=0)
    return out.astype(np.float32), res


def kernel(**inputs) -> np.ndarray:
    out, _ = _run(inputs, trace=False)
    return out


# ------------------------------------------------- bench (timing) harness

def _bench(inputs, iters=24):
    """Return (out, per_call_ns, single_ns) via steady-state async enqueue."""
    import sys, time
    if "/opt/trn_rl_repo" not in sys.path:
        sys.path.insert(0, "/opt/trn_rl_repo")
    _pin_neff_cache()
    import jax
    from jax.experimental.shard_map import shard_map
    from jax.sharding import Mesh, PartitionSpec
    from concourse import bass2jax, mybir
    from concourse.bass2jax import _bass_exec_p, partition_id_tensor

    bass2jax.install_neuronx_cc_hook()
    in_maps, (NBW, NPW, NPAIR, NCHK), (use_b1, use_b2) = _make_in_maps(inputs)
    nc = _build(NBW, NPW, NPAIR, NCHK, use_b1=use_b1, use_b2=use_b2)

    in_names, out_names, out_avals, zero_outs = [], [], [], []
    for alloc in nc.m.functions[0].allocations:
        if not isinstance(alloc, mybir.MemoryLocationSet):
            continue
        name = alloc.memorylocations[0].name
        if alloc.kind == "ExternalInput":
            if nc.partition_id_tensor is None or name != nc.partition_id_tensor.name:
                in_names.append(name)
        elif alloc.kind == "ExternalOutput":
            shape = tuple(alloc.tensor_shape)
            dtype = mybir.dt.np(alloc.dtype)
            out_names.append(name)
            out_avals.append(jax.core.ShapedArray(shape, dtype))
            zero_outs.append(np.zeros(shape, dtype))
    n_params = len(in_names)
    all_names = list(in_names) + out_names
    pname = nc.partition_id_tensor.name if nc.partition_id_tensor else None
    if pname is not None:
        all_names.append(pname)

    def _body(*args):
        operands = list(args)
        if pname is not None:
            operands.append(partition_id_tensor())
        return tuple(_bass_exec_p.bind(
            *operands, out_avals=tuple(out_avals), in_names=tuple(all_names),
            out_names=tuple(out_names), lowering_input_output_aliases=(),
            sim_require_finite=True, sim_require_nnan=True, nc=nc))

    devices = jax.devices()[:NCORES]
    mesh = Mesh(np.asarray(devices), ("core",))
    nio = n_params + len(out_names)
    fn = jax.jit(shard_map(_body, mesh=mesh,
                           in_specs=(PartitionSpec("core"),) * nio,
                           out_specs=(PartitionSpec("core"),) * len(out_names),
                           check_rep=False), keep_unused=True)
    concat_in = [np.concatenate([np.asarray(in_maps[c][nm])[None]
                                 for c in range(NCORES)], axis=0)
                 .reshape(-1, *np.asarray(in_maps[0][nm]).shape[1:])
                 for nm in in_names]
    concat_zero = [np.concatenate([z[None]] * NCORES, axis=0)
                   .reshape(-1, *z.shape[1:]) for z in zero_outs]
    dev_in = [jax.device_put(a) for a in concat_in + concat_zero]
    outs = fn(*dev_in)
    jax.block_until_ready(outs)
    t0 = time.perf_counter()
    outs = fn(*dev_in)
    jax.block_until_ready(outs)
    one = time.perf_counter() - t0
    t0 = time.perf_counter()
    last = None
    for _ in range(iters):
        last = fn(*dev_in)
    jax.block_until_ready(last)
    per = (time.perf_counter() - t0) / iters
    out_full = 2.0 * np.concatenate(
        [np.asarray(outs[0]).reshape(NCORES, GPC, D)[c] for c in range(NCORES)],
        axis=0)
    return out_full.astype(np.float32), per * 1e9, one * 1e9
